# revision 10
# baseline (speedup 1.0000x reference)
"""Trainium2 Bass kernel for nn_ContextAttention_21457656611319.

Reference math (per batch n):
    xf = x[n] reshaped [C, L], L = H*W = 4096
    q = Wq@xf + bq ; k = Wk@xf + bk ; v = Wv@xf + bv          [C, L]
    S[l,m] = sum_c k[c,l] q[c,m] * (1/sqrt(C))                 [L, L]
    T = softmax(S, axis=m)
    attn[c,m] = sum_l v[c,l] T[l,m]
    out = x + attn

Sharding: 8 cores = 4 batches x 2-way shard of the l (key/value) axis.
Each core computes a partial attn over its l-half; the host adds the
two halves per batch plus x.  No collectives.

Per-core plan (l-half LH=2048 -> 16 l-tiles of 128).  The softmax row
scale is absorbed into v (vts = v/Z), so T is stored unnormalized.
Work is spread over all four engines:

  PE:   S chunks (bf16), attn matmuls: m-chunks 0-2 as fp8 DoubleRow
        over l-tile PAIRS (2 k-tiles per pass), m-chunk 3 in bf16.
  ACT:  exp for chunks 0-2 -> fp8e4 (T scaled by 1/4 via exp bias:
        fp8e4 max is 240 and raw exp reaches ~380), accum_out riding
        chunks 0-1 for their row sums.
  DVE:  exp for chunk 3 via the Schraudolph bit trick (one
        tensor_scalar: i16 = A*s + B, bitcast to bf16), z-combine /
        reciprocal / vts scaling, PSUM->SBUF evacuations, attn drains.
  Pool: row sums of chunks 2 (fp8) and 3 (bf16) via tensor_scalar
        with accum_out into a dummy buffer.

vts for the fp8 passes is scaled by kappa=1024 (v/Z ~ 5e-4 underflows
fp8's subnormal floor); the attn drain multiplies by 1/kappa.
"""

import sys

if "/opt/trn_rl_repo" not in sys.path:
    sys.path.insert(0, "/opt/trn_rl_repo")

import numpy as np

N, C, H, W = 4, 128, 64, 64
L = H * W            # 4096
LH = L // 2          # 2048 l-half per core
P = 128              # partitions / l-tile size
NT = LH // P         # 16 l-tiles per core
BANK = 512           # fp32 elems per PSUM bank
CH = 1024            # S-chunk / attn-accumulator width (2 PSUM banks)
NCH = L // CH        # 4 chunks
NCORES = 8
SCALE = float(1.0 / np.sqrt(C))
LN4 = float(np.log(4.0))
KAPPA = 1024.0
# Schraudolph exp in bf16 bits: i16 = trunc(A*scale*s + B), bitcast bf16.
# B = 127*128 + 0.5 (trunc->round) - 7.5 (rel-err balance) - 256 (the 1/4
# T scaling, to match the fp8 chunks' exp bias).
SCH_A = float(2.0 ** 7 / np.log(2.0) * SCALE)
SCH_B = 127.0 * 128.0 + 0.5 - 7.5 - 256.0

_CACHE = {}


def _build_nc():
    import concourse.bass as bass
    import concourse.tile as tile
    from concourse import bacc, mybir
    from contextlib import ExitStack

    f32 = mybir.dt.float32
    bf16 = mybir.dt.bfloat16
    f8 = mybir.dt.float8e4
    i16 = mybir.dt.int16
    DR = mybir.MatmulPerfMode.DoubleRow
    Mul = mybir.AluOpType.mult
    Add = mybir.AluOpType.add

    nc = bacc.Bacc("TRN2", target_bir_lowering=False, debug=False)

    xf = nc.dram_tensor("xf", [P, L], bf16, kind="ExternalInput").ap()
    xh = nc.dram_tensor("xh", [P, LH], bf16, kind="ExternalInput").ap()
    wqT = nc.dram_tensor("wqT", [P, P], bf16, kind="ExternalInput").ap()
    wkT = nc.dram_tensor("wkT", [P, P], bf16, kind="ExternalInput").ap()
    wvT = nc.dram_tensor("wvT", [P, P], bf16, kind="ExternalInput").ap()
    bq = nc.dram_tensor("bq", [P, 1], f32, kind="ExternalInput").ap()
    bk = nc.dram_tensor("bk", [P, 1], f32, kind="ExternalInput").ap()
    bv = nc.dram_tensor("bv", [1, P], f32, kind="ExternalInput").ap()
    attn_out = nc.dram_tensor("attn_part", [P, L], f32, kind="ExternalOutput").ap()

    Exp = mybir.ActivationFunctionType.Exp

    with tile.TileContext(nc) as tc, ExitStack() as ctx:
        const = ctx.enter_context(tc.tile_pool(name="const", bufs=1))
        persist = ctx.enter_context(tc.tile_pool(name="persist", bufs=1))

        wq_sb = const.tile([P, P], bf16)
        wk_sb = const.tile([P, P], bf16)
        wv_sb = const.tile([P, P], bf16)
        bq_sb = const.tile([P, 1], f32)
        bk_sb = const.tile([P, 1], f32)
        bv_sb = const.tile([P, P], f32)  # bv broadcast across partitions
        warm = const.tile([P, 1], f32)
        lnb = const.tile([P, 1], f32)
        nc.vector.memset(lnb, -LN4)
        nc.scalar.dma_start(out=bk_sb, in_=bk)
        nc.scalar.dma_start(out=wk_sb, in_=wkT)
        nc.scalar.dma_start(out=wq_sb, in_=wqT)
        nc.scalar.dma_start(out=bq_sb, in_=bq)
        nc.scalar.dma_start(out=wv_sb, in_=wvT)
        bv_bcast = bass.AP(tensor=bv.tensor, offset=bv.offset,
                           ap=[[0, P], bv.ap[1]])
        nc.scalar.dma_start(out=bv_sb, in_=bv_bcast)
        # warm the ACT exp table while DMAs run
        nc.scalar.activation(warm, bk_sb, Exp, scale=0.0)

        q_sb = persist.tile([P, L], bf16)
        k_sb = persist.tile([P, LH], bf16)
        vt_sb = persist.tile([P, NT, P], f32)    # [l, tile, c]
        vts8 = persist.tile([P, NT, P], f8)      # v * kappa/Z, fp8
        vts16 = persist.tile([P, NT, P], bf16)   # v / Z, bf16
        z4 = persist.tile([P, NT, NCH], f32)     # per-chunk row sums of T
        zs = persist.tile([P, NT], f32)
        rs = persist.tile([P, NT], f32)
        attn_sb = persist.tile([P, L], f32)      # attn partial accumulator

        # fp8 T chunks 0-2 and bf16 T chunk 3 (Schraudolph), kept apart so
        # the fp8 region stays contiguous for the DoubleRow moving APs.
        t8 = persist.tile([P, NT, 3 * CH], f8)
        t3 = persist.tile([P, NT, CH], bf16)

        with tc.tile_pool(name="sps", bufs=3, space="PSUM") as sp, \
             tc.tile_pool(name="aps", bufs=1, space="PSUM") as ap, \
             tc.tile_pool(name="outp", bufs=2) as outp, \
             tc.tile_pool(name="zfp", bufs=2) as zfp, \
             tc.tile_pool(name="xp", bufs=1) as xp:

            x_sb = xp.tile([P, L], bf16)
            xh_sb = xp.tile([P, LH], bf16)
            for j in range(4):
                nc.sync.dma_start(out=xh_sb[:, j * BANK:(j + 1) * BANK],
                                  in_=xh[:, j * BANK:(j + 1) * BANK])
            for h in range(4):
                msl = slice(h * CH, (h + 1) * CH)
                eng = nc.sync if h % 2 else nc.scalar
                eng.dma_start(out=x_sb[:, msl], in_=xf[:, msl])

            def q_pass(h):
                t = sp.tile([P, CH], f32, tag="s", name="qp")
                for j in range(CH // BANK):
                    c0 = h * CH + j * BANK
                    nc.tensor.matmul(t[:, j * BANK:(j + 1) * BANK],
                                     wq_sb, x_sb[:, c0:c0 + BANK])
                nc.vector.tensor_scalar_add(q_sb[:, h * CH:(h + 1) * CH],
                                            t, bq_sb)

            def k_pass(h):
                t = sp.tile([P, CH], f32, tag="s", name="kp")
                for j in range(CH // BANK):
                    c0 = h * CH + j * BANK
                    nc.tensor.matmul(t[:, j * BANK:(j + 1) * BANK],
                                     wk_sb, xh_sb[:, c0:c0 + BANK])
                nc.vector.tensor_scalar_add(k_sb[:, h * CH:(h + 1) * CH],
                                            t, bk_sb)

            def vt_pass(h):
                t = ap.tile([P, CH // P, P], f32, tag="acc", name="vtp")
                for j in range(CH // P):
                    i = h * (CH // P) + j
                    nc.tensor.matmul(t[:, j, :],
                                     xh_sb[:, i * P:(i + 1) * P], wv_sb)
                bvb = bv_sb[:, :]
                bvb = bass.AP(tensor=bvb.tensor, offset=bvb.offset,
                              ap=[bvb.ap[0], [0, CH // P], bvb.ap[1]])
                nc.vector.tensor_add(
                    vt_sb[:, h * (CH // P):(h + 1) * (CH // P), :], t, bvb)

            # ---- attn sub-passes ---------------------------------------
            # m-chunks 0-2: fp8 DoubleRow over l-tile pairs, grouped.
            # m-chunk 3:    bf16 per-tile matmuls, grouped (baseline style).
            FP8_GROUPS = [(0, 3), (3, 3), (6, 1), (7, 1)]  # (first pair, npairs)
            BF16_GROUPS = [(0, 6), (6, 6), (12, 2), (14, 2)]  # (first tile, n)

            def fp8_pass(g, sub):
                g0, glen = FP8_GROUPS[g]
                t = ap.tile([P, CH], f32, tag="acc", name="acc")
                for idx in range(glen):
                    i2 = 2 * (g0 + idx)
                    for hh in range(2):
                        m0 = sub * CH + hh * BANK
                        nc.tensor.matmul(
                            t[:, hh * BANK:(hh + 1) * BANK],
                            vts8[:, i2:i2 + 2, :],
                            t8[:, i2:i2 + 2, m0:m0 + BANK],
                            start=(idx == 0), stop=(idx == glen - 1),
                            perf_mode=DR)
                _drain(t, sub, g == 0, g == len(FP8_GROUPS) - 1, 1.0 / KAPPA)

            def bf16_pass(g):
                g0, glen = BF16_GROUPS[g]
                t = ap.tile([P, CH], f32, tag="acc", name="acc")
                for idx in range(glen):
                    i = g0 + idx
                    for hh in range(2):
                        m0 = hh * BANK
                        nc.tensor.matmul(
                            t[:, hh * BANK:(hh + 1) * BANK],
                            vts16[:, i, :],
                            t3[:, i, m0:m0 + BANK],
                            start=(idx == 0), stop=(idx == glen - 1))
                _drain(t, 3, g == 0, g == len(BF16_GROUPS) - 1, 1.0)

            def _drain(t, sub, first, last, fac):
                msl = slice(sub * CH, (sub + 1) * CH)
                if first:
                    nc.vector.tensor_scalar_mul(attn_sb[:, msl], t, fac)
                elif not last:
                    nc.vector.scalar_tensor_tensor(
                        attn_sb[:, msl], t, fac, attn_sb[:, msl],
                        op0=Mul, op1=Add)
                else:
                    ao = outp.tile([P, CH], f32, tag="ao", name="ao")
                    nc.vector.scalar_tensor_tensor(
                        ao, t, fac, attn_sb[:, msl], op0=Mul, op1=Add)
                    nc.sync.dma_start(out=attn_out[:, msl], in_=ao)

            # pass schedule: tile index -> list of thunk keys
            sched = {i: [] for i in range(NT)}
            sched[7] = [("f", 0, 0)]
            sched[8] = [("f", 0, 1), ("b", 0)]
            sched[9] = [("f", 0, 2)]
            sched[12] = [("f", 1, 0)]
            sched[13] = [("f", 1, 1), ("b", 1)]
            sched[14] = [("f", 1, 2), ("f", 2, 0)]
            sched[15] = [("f", 2, 1), ("b", 2)]
            tail = [("f", 2, 2), ("f", 3, 0), ("f", 3, 1), ("f", 3, 2),
                    ("b", 3)]

            def run_pass(p):
                if p[0] == "f":
                    fp8_pass(p[1], p[2])
                else:
                    bf16_pass(p[1])

            def s_chunk(i, c):
                s = sp.tile([P, CH], f32, tag="s")
                for j in range(CH // BANK):
                    m0 = c * CH + j * BANK
                    nc.tensor.matmul(s[:, j * BANK:(j + 1) * BANK],
                                     k_sb[:, i * P:(i + 1) * P],
                                     q_sb[:, m0:m0 + BANK])
                if c < 3:
                    nc.scalar.activation(
                        t8[:, i, c * CH:(c + 1) * CH], s, Exp, scale=SCALE,
                        bias=lnb[:, :],
                        accum_out=z4[:, i, c:c + 1])
                else:
                    nc.vector.tensor_scalar(
                        t3[:, i, :].bitcast(i16), s, SCH_A, SCH_B,
                        op0=Mul, op1=Add)
                    zf = zfp.tile([P, BANK], bf16, tag="zf")
                    nc.gpsimd.tensor_add(zf, t3[:, i, :BANK], t3[:, i, BANK:])
                    nc.vector.reduce_sum(out=z4[:, i, 3:4], in_=zf,
                                         axis=mybir.AxisListType.X)

            def ztail(i):
                nc.vector.reduce_sum(out=zs[:, i:i + 1], in_=z4[:, i, :],
                                     axis=mybir.AxisListType.X)
                nc.vector.reciprocal(rs[:, i:i + 1], zs[:, i:i + 1])
                nc.vector.tensor_scalar(vts8[:, i, :], vt_sb[:, i, :],
                                        rs[:, i:i + 1], KAPPA,
                                        op0=Mul, op1=Mul)
                nc.vector.tensor_scalar_mul(vts16[:, i, :], vt_sb[:, i, :],
                                            rs[:, i:i + 1])

            # ---- emission: projections woven into the first tiles ------
            k_pass(0); k_pass(1)
            q_pass(0)
            vt_pass(0)
            for i in range(NT):
                if i == 0:
                    q_pass(1)
                s_chunk(i, 0)
                if i == 0:
                    q_pass(2)
                s_chunk(i, 1)
                if i == 0:
                    q_pass(3)
                if i == 1:
                    vt_pass(1)
                for p in sched[i][:1]:
                    run_pass(p)
                s_chunk(i, 2)
                for p in sched[i][1:]:
                    run_pass(p)
                s_chunk(i, 3)
                ztail(i)
            for p in tail:
                run_pass(p)

    nc.compile()
    return nc


def _get_nc():
    if "nc" not in _CACHE:
        _CACHE["nc"] = _build_nc()
    return _CACHE["nc"]


def _make_in_maps(inputs):
    import ml_dtypes
    bf = ml_dtypes.bfloat16
    x = np.ascontiguousarray(np.asarray(inputs["x"], dtype=np.float32))
    wqT = np.ascontiguousarray(np.asarray(inputs["Wq"], dtype=np.float32).T.astype(bf))
    wkT = np.ascontiguousarray(np.asarray(inputs["Wk"], dtype=np.float32).T.astype(bf))
    wvT = np.ascontiguousarray(np.asarray(inputs["Wv"], dtype=np.float32).T.astype(bf))
    bq = np.ascontiguousarray(np.asarray(inputs["bq"], dtype=np.float32).reshape(P, 1))
    bk = np.ascontiguousarray(np.asarray(inputs["bk"], dtype=np.float32).reshape(P, 1))
    bv = np.ascontiguousarray(np.asarray(inputs["bv"], dtype=np.float32).reshape(1, P))
    in_maps = []
    for core in range(NCORES):
        n, half = core // 2, core % 2
        xf32 = x[n].reshape(C, L)
        xfb = np.ascontiguousarray(xf32.astype(bf))
        xhb = np.ascontiguousarray(xfb[:, half * LH:(half + 1) * LH])
        in_maps.append({
            "xf": xfb, "xh": xhb,
            "wqT": wqT, "wkT": wkT, "wvT": wvT,
            "bq": bq, "bk": bk, "bv": bv,
        })
    return in_maps, x


def run_on_hw(inputs, trace=False, **kwargs):
    """Returns (list of per-core attn_part arrays, BassKernelResults)."""
    from concourse import bass_utils
    nc = _get_nc()
    in_maps, _ = _make_in_maps(inputs)
    res = bass_utils.run_bass_kernel_spmd(
        nc, in_maps, list(range(NCORES)), trace=trace, **kwargs)
    parts = [res.results[i]["attn_part"] for i in range(NCORES)]
    return parts, res


def kernel(**inputs) -> np.ndarray:
    in_maps, x = _make_in_maps(inputs)
    parts, _ = run_on_hw(inputs)
    out = np.empty((N, C, H, W), dtype=np.float32)
    for n in range(N):
        attn = parts[2 * n] + parts[2 * n + 1]
        out[n] = x[n] + attn.reshape(C, H, W)
    return out


# revision 11
# speedup vs baseline: 1.1124x; 1.1124x over previous
"""Trainium2 Bass kernel for nn_ContextAttention_21457656611319.

Reference math (per batch n):
    xf = x[n] reshaped [C, L], L = H*W = 4096
    q = Wq@xf + bq ; k = Wk@xf + bk ; v = Wv@xf + bv          [C, L]
    S[l,m] = sum_c k[c,l] q[c,m] * (1/sqrt(C))                 [L, L]
    T = softmax(S, axis=m)
    attn[c,m] = sum_l v[c,l] T[l,m]
    out = x + attn

Sharding: 8 cores = 4 batches x 2-way shard of the l (key/value) axis.
Each core computes a partial attn over its l-half; the host adds the
two halves per batch plus x.  No collectives.

Per-core plan (l-half LH=2048 -> 16 l-tiles of 128).  The softmax row
scale is absorbed into v (vts = v/Z), so T is stored unnormalized.
Work is spread over all four engines:

  PE:   S chunks (bf16), attn matmuls: m-chunks 0-2 as fp8 DoubleRow
        over l-tile PAIRS (2 k-tiles per pass), m-chunk 3 in bf16.
  ACT:  exp for chunks 0-2 -> fp8e4 (T scaled by 1/4 via exp bias:
        fp8e4 max is 240 and raw exp reaches ~380), accum_out riding
        chunks 0-1 for their row sums.
  DVE:  exp for chunk 3 via the Schraudolph bit trick (one
        tensor_scalar: i16 = A*s + B, bitcast to bf16), z-combine /
        reciprocal / vts scaling, PSUM->SBUF evacuations, attn drains.
  Pool: row sums of chunks 2 (fp8) and 3 (bf16) via tensor_scalar
        with accum_out into a dummy buffer.

vts for the fp8 passes is scaled by kappa=1024 (v/Z ~ 5e-4 underflows
fp8's subnormal floor); the attn drain multiplies by 1/kappa.
"""

import sys

if "/opt/trn_rl_repo" not in sys.path:
    sys.path.insert(0, "/opt/trn_rl_repo")

import numpy as np

N, C, H, W = 4, 128, 64, 64
L = H * W            # 4096
LH = L // 2          # 2048 l-half per core
P = 128              # partitions / l-tile size
NT = LH // P         # 16 l-tiles per core
BANK = 512           # fp32 elems per PSUM bank
CH = 1024            # S-chunk / attn-accumulator width (2 PSUM banks)
NCH = L // CH        # 4 chunks
NCORES = 8
SCALE = float(1.0 / np.sqrt(C))
LN4 = float(np.log(4.0))
KAPPA = 1024.0
# Schraudolph exp in bf16 bits: i16 = trunc(A*scale*s + B), bitcast bf16.
# B = 127*128 + 0.5 (trunc->round) - 7.5 (rel-err balance) - 256 (the 1/4
# T scaling, to match the fp8 chunks' exp bias).
SCH_A = float(2.0 ** 7 / np.log(2.0) * SCALE)
SCH_B = 127.0 * 128.0 + 0.5 - 7.5 - 256.0

_CACHE = {}


def _build_nc():
    import concourse.bass as bass
    import concourse.tile as tile
    from concourse import bacc, mybir
    from contextlib import ExitStack

    f32 = mybir.dt.float32
    bf16 = mybir.dt.bfloat16
    f8 = mybir.dt.float8e4
    i16 = mybir.dt.int16
    DR = mybir.MatmulPerfMode.DoubleRow
    Mul = mybir.AluOpType.mult
    Add = mybir.AluOpType.add

    nc = bacc.Bacc("TRN2", target_bir_lowering=False, debug=False)

    xf = nc.dram_tensor("xf", [P, L], bf16, kind="ExternalInput").ap()
    xh = nc.dram_tensor("xh", [P, LH], bf16, kind="ExternalInput").ap()
    wqT = nc.dram_tensor("wqT", [P, P], bf16, kind="ExternalInput").ap()
    wkT = nc.dram_tensor("wkT", [P, P], bf16, kind="ExternalInput").ap()
    wvT = nc.dram_tensor("wvT", [P, P], bf16, kind="ExternalInput").ap()
    bq = nc.dram_tensor("bq", [P, 1], f32, kind="ExternalInput").ap()
    bk = nc.dram_tensor("bk", [P, 1], f32, kind="ExternalInput").ap()
    bv = nc.dram_tensor("bv", [1, P], f32, kind="ExternalInput").ap()
    attn_out = nc.dram_tensor("attn_part", [P, L], f32, kind="ExternalOutput").ap()

    Exp = mybir.ActivationFunctionType.Exp

    with tile.TileContext(nc) as tc, ExitStack() as ctx:
        const = ctx.enter_context(tc.tile_pool(name="const", bufs=1))
        persist = ctx.enter_context(tc.tile_pool(name="persist", bufs=1))

        wq_sb = const.tile([P, P], bf16)
        wk_sb = const.tile([P, P], bf16)
        wv_sb = const.tile([P, P], bf16)
        bq_sb = const.tile([P, 1], f32)
        bk_sb = const.tile([P, 1], f32)
        bv_sb = const.tile([P, P], f32)  # bv broadcast across partitions
        warm = const.tile([P, 1], f32)
        lnb = const.tile([P, 1], f32)
        nc.vector.memset(lnb, -LN4)
        nc.sync.dma_start(out=bk_sb, in_=bk)
        nc.sync.dma_start(out=wk_sb, in_=wkT)
        nc.sync.dma_start(out=wq_sb, in_=wqT)
        nc.sync.dma_start(out=bq_sb, in_=bq)
        nc.sync.dma_start(out=wv_sb, in_=wvT)
        bv_bcast = bass.AP(tensor=bv.tensor, offset=bv.offset,
                           ap=[[0, P], bv.ap[1]])
        nc.sync.dma_start(out=bv_sb, in_=bv_bcast)
        # warm the ACT exp table while DMAs run
        nc.scalar.activation(warm, bk_sb, Exp, scale=0.0)

        q_sb = persist.tile([P, L], bf16)
        k_sb = persist.tile([P, LH], bf16)
        vt_sb = persist.tile([P, NT, P], f32)    # [l, tile, c]
        vts8 = persist.tile([P, NT, P], f8)      # v * kappa/Z, fp8
        vts16 = persist.tile([P, NT, P], bf16)   # v / Z, bf16
        z4 = persist.tile([P, NT, NCH], f32)     # per-chunk row sums of T
        zs = persist.tile([P, NT], f32)
        rs = persist.tile([P, NT], f32)
        attn_sb = persist.tile([P, L], f32)      # attn partial accumulator

        # fp8 T chunks 0-2 and bf16 T chunk 3 (Schraudolph), kept apart so
        # the fp8 region stays contiguous for the DoubleRow moving APs.
        t8 = persist.tile([P, NT, 3 * CH], f8)
        t3 = persist.tile([P, NT, CH], bf16)

        with tc.tile_pool(name="sps", bufs=3, space="PSUM") as sp, \
             tc.tile_pool(name="aps", bufs=1, space="PSUM") as ap, \
             tc.tile_pool(name="outp", bufs=2) as outp, \
             tc.tile_pool(name="xp", bufs=1) as xp:

            x_sb = xp.tile([P, L], bf16)
            xh_sb = xp.tile([P, LH], bf16)
            for j in range(4):
                nc.sync.dma_start(out=xh_sb[:, j * BANK:(j + 1) * BANK],
                                  in_=xh[:, j * BANK:(j + 1) * BANK])
            for h in range(4):
                msl = slice(h * CH, (h + 1) * CH)
                nc.sync.dma_start(out=x_sb[:, msl], in_=xf[:, msl])

            def q_pass(h):
                t = sp.tile([P, CH], f32, tag="s", name="qp")
                for j in range(CH // BANK):
                    c0 = h * CH + j * BANK
                    nc.tensor.matmul(t[:, j * BANK:(j + 1) * BANK],
                                     wq_sb, x_sb[:, c0:c0 + BANK])
                nc.vector.tensor_scalar_add(q_sb[:, h * CH:(h + 1) * CH],
                                            t, bq_sb)

            def k_pass(h):
                t = sp.tile([P, CH], f32, tag="s", name="kp")
                for j in range(CH // BANK):
                    c0 = h * CH + j * BANK
                    nc.tensor.matmul(t[:, j * BANK:(j + 1) * BANK],
                                     wk_sb, xh_sb[:, c0:c0 + BANK])
                nc.vector.tensor_scalar_add(k_sb[:, h * CH:(h + 1) * CH],
                                            t, bk_sb)

            def vt_pass(h):
                t = ap.tile([P, CH // P, P], f32, tag="acc", name="vtp")
                for j in range(CH // P):
                    i = h * (CH // P) + j
                    nc.tensor.matmul(t[:, j, :],
                                     xh_sb[:, i * P:(i + 1) * P], wv_sb)
                bvb = bv_sb[:, :]
                bvb = bass.AP(tensor=bvb.tensor, offset=bvb.offset,
                              ap=[bvb.ap[0], [0, CH // P], bvb.ap[1]])
                nc.vector.tensor_add(
                    vt_sb[:, h * (CH // P):(h + 1) * (CH // P), :], t, bvb)

            # ---- attn sub-passes ---------------------------------------
            # m-chunks 0-2: fp8 DoubleRow over l-tile pairs, grouped.
            # m-chunk 3:    bf16 per-tile matmuls, grouped (baseline style).
            FP8_GROUPS = [(0, 3), (3, 3), (6, 1), (7, 1)]  # (first pair, npairs)
            BF16_GROUPS = [(0, 6), (6, 6), (12, 2), (14, 2)]  # (first tile, n)

            def fp8_pass(g, sub):
                g0, glen = FP8_GROUPS[g]
                t = ap.tile([P, CH], f32, tag="acc", name="acc")
                for idx in range(glen):
                    i2 = 2 * (g0 + idx)
                    for hh in range(2):
                        m0 = sub * CH + hh * BANK
                        nc.tensor.matmul(
                            t[:, hh * BANK:(hh + 1) * BANK],
                            vts8[:, i2:i2 + 2, :],
                            t8[:, i2:i2 + 2, m0:m0 + BANK],
                            start=(idx == 0), stop=(idx == glen - 1),
                            perf_mode=DR)
                _drain(t, sub, g == 0, g == len(FP8_GROUPS) - 1, 1.0 / KAPPA)

            def bf16_pass(g):
                g0, glen = BF16_GROUPS[g]
                t = ap.tile([P, CH], f32, tag="acc", name="acc")
                for idx in range(glen):
                    i = g0 + idx
                    for hh in range(2):
                        m0 = hh * BANK
                        nc.tensor.matmul(
                            t[:, hh * BANK:(hh + 1) * BANK],
                            vts16[:, i, :],
                            t3[:, i, m0:m0 + BANK],
                            start=(idx == 0), stop=(idx == glen - 1))
                _drain(t, 3, g == 0, g == len(BF16_GROUPS) - 1, 1.0)

            def _drain(t, sub, first, last, fac):
                msl = slice(sub * CH, (sub + 1) * CH)
                if first:
                    nc.vector.tensor_scalar_mul(attn_sb[:, msl], t, fac)
                elif not last:
                    nc.vector.scalar_tensor_tensor(
                        attn_sb[:, msl], t, fac, attn_sb[:, msl],
                        op0=Mul, op1=Add)
                else:
                    ao = outp.tile([P, CH], f32, tag="ao", name="ao")
                    nc.vector.scalar_tensor_tensor(
                        ao, t, fac, attn_sb[:, msl], op0=Mul, op1=Add)
                    nc.sync.dma_start(out=attn_out[:, msl], in_=ao)

            # pass schedule: tile index -> list of thunk keys
            sched = {i: [] for i in range(NT)}
            sched[7] = [("f", 0, 0)]
            sched[8] = [("f", 0, 1), ("b", 0)]
            sched[9] = [("f", 0, 2)]
            sched[12] = [("f", 1, 0)]
            sched[13] = [("f", 1, 1), ("b", 1)]
            sched[14] = [("f", 1, 2), ("f", 2, 0)]
            sched[15] = [("f", 2, 1), ("b", 2)]
            tail = [("f", 2, 2), ("f", 3, 0), ("f", 3, 1), ("f", 3, 2),
                    ("b", 3)]

            def run_pass(p):
                if p[0] == "f":
                    fp8_pass(p[1], p[2])
                else:
                    bf16_pass(p[1])

            def s_chunk(i, c):
                s = sp.tile([P, CH], f32, tag="s")
                for j in range(CH // BANK):
                    m0 = c * CH + j * BANK
                    nc.tensor.matmul(s[:, j * BANK:(j + 1) * BANK],
                                     k_sb[:, i * P:(i + 1) * P],
                                     q_sb[:, m0:m0 + BANK])
                if c < 3:
                    nc.scalar.activation(
                        t8[:, i, c * CH:(c + 1) * CH], s, Exp, scale=SCALE,
                        bias=lnb[:, :],
                        accum_out=z4[:, i, c:c + 1])
                else:
                    nc.vector.tensor_scalar(
                        t3[:, i, :].bitcast(i16), s, SCH_A, SCH_B,
                        op0=Mul, op1=Add)
                    nc.vector.reduce_sum(out=z4[:, i, 3:4], in_=t3[:, i, :],
                                         axis=mybir.AxisListType.X)

            def ztail(i):
                nc.vector.reduce_sum(out=zs[:, i:i + 1], in_=z4[:, i, :],
                                     axis=mybir.AxisListType.X)
                nc.vector.reciprocal(rs[:, i:i + 1], zs[:, i:i + 1])
                nc.vector.tensor_scalar(vts8[:, i, :], vt_sb[:, i, :],
                                        rs[:, i:i + 1], KAPPA,
                                        op0=Mul, op1=Mul)
                nc.vector.tensor_scalar_mul(vts16[:, i, :], vt_sb[:, i, :],
                                            rs[:, i:i + 1])

            # ---- emission: projections woven into the first tiles ------
            k_pass(0); k_pass(1)
            q_pass(0)
            vt_pass(0)
            for i in range(NT):
                if i == 0:
                    q_pass(1)
                s_chunk(i, 0)
                if i == 0:
                    q_pass(2)
                s_chunk(i, 1)
                if i == 0:
                    q_pass(3)
                if i == 1:
                    vt_pass(1)
                for p in sched[i][:1]:
                    run_pass(p)
                s_chunk(i, 2)
                for p in sched[i][1:]:
                    run_pass(p)
                s_chunk(i, 3)
                ztail(i)
            for p in tail:
                run_pass(p)

    nc.compile()
    return nc


def _get_nc():
    if "nc" not in _CACHE:
        _CACHE["nc"] = _build_nc()
    return _CACHE["nc"]


def _make_in_maps(inputs):
    import ml_dtypes
    bf = ml_dtypes.bfloat16
    x = np.ascontiguousarray(np.asarray(inputs["x"], dtype=np.float32))
    wqT = np.ascontiguousarray(np.asarray(inputs["Wq"], dtype=np.float32).T.astype(bf))
    wkT = np.ascontiguousarray(np.asarray(inputs["Wk"], dtype=np.float32).T.astype(bf))
    wvT = np.ascontiguousarray(np.asarray(inputs["Wv"], dtype=np.float32).T.astype(bf))
    bq = np.ascontiguousarray(np.asarray(inputs["bq"], dtype=np.float32).reshape(P, 1))
    bk = np.ascontiguousarray(np.asarray(inputs["bk"], dtype=np.float32).reshape(P, 1))
    bv = np.ascontiguousarray(np.asarray(inputs["bv"], dtype=np.float32).reshape(1, P))
    in_maps = []
    for core in range(NCORES):
        n, half = core // 2, core % 2
        xf32 = x[n].reshape(C, L)
        xfb = np.ascontiguousarray(xf32.astype(bf))
        xhb = np.ascontiguousarray(xfb[:, half * LH:(half + 1) * LH])
        in_maps.append({
            "xf": xfb, "xh": xhb,
            "wqT": wqT, "wkT": wkT, "wvT": wvT,
            "bq": bq, "bk": bk, "bv": bv,
        })
    return in_maps, x


def run_on_hw(inputs, trace=False, **kwargs):
    """Returns (list of per-core attn_part arrays, BassKernelResults)."""
    from concourse import bass_utils
    nc = _get_nc()
    in_maps, _ = _make_in_maps(inputs)
    res = bass_utils.run_bass_kernel_spmd(
        nc, in_maps, list(range(NCORES)), trace=trace, **kwargs)
    parts = [res.results[i]["attn_part"] for i in range(NCORES)]
    return parts, res


def kernel(**inputs) -> np.ndarray:
    in_maps, x = _make_in_maps(inputs)
    parts, _ = run_on_hw(inputs)
    out = np.empty((N, C, H, W), dtype=np.float32)
    for n in range(N):
        attn = parts[2 * n] + parts[2 * n + 1]
        out[n] = x[n] + attn.reshape(C, H, W)
    return out
